# revision 2
# baseline (speedup 1.0000x reference)
"""CP-decomposed 3x3 conv on 8 TRN2 NeuronCores — v2 (K=128-packed).

Math: out[f,i,j] = sum_{h,w,c,r} in[c,i+h,j+w] * f1[h,r] * f2[w,r] * f3[c,r] * f0[f,r]

Per-core factorization (32 output rows each; n flattens (row,col) with pitch
W=256; cols 254/255 of each row are garbage, trimmed on host):

  stage A (2 matmuls per 512-col chunk):
    mmA1 (K=128): lhsT=[wa0;wa1], rhs=XA[0:128, n]
                  where XA lower = x[c,n], upper = x[c,n+W] (host-shifted)
    mmA2 (K=64):  lhsT=wa2,       rhs=XA[0:64, n+2W]          (accumulate)
      -> t2[r, n] in PSUM
  evac: PSUM fp32 -> SBUF bf16 T2P pair tile (2 chunks, 520-col slots)
  dup: one SBUF->SBUF DMA per pair: T2P upper half = t2 shifted +1 col
  stage B (2 matmuls per chunk):
    mmB1 (K=128): lhsT=[wb0;wb1], rhs=T2P[0:128, slot]
    mmB2 (K=64):  lhsT=wb2,       rhs=T2P[0:64, slot+2]       (accumulate)
  out copy: PSUM fp32 -> SBUF bf16; one store DMA per pair -> y (bf16).

All inputs ride in ONE host-packed bf16 tensor xall [128, 9152]:
  cols [0,320):   W1 = [[wa0;wa1] | [wb0;wb1] | [wa2;0] | unused]
  cols [320,448): W2 = [wb2 ; 0]
  cols [448,...): lower rows = x (34 rows flat), upper rows = x shifted
                  one row (33 rows flat + zero pad)
where wa_h[c,r]=f3[c,r]*f1[h,r], wb_w[r,f]=f2[w,r]*f0[f,r] (host-packed).

Sharding: output rows (Ho=254): cores 0-6 rows [32i,32i+32), core 7 rows
[222,254) (first 2 dup'd, dropped at gather). Host upcasts y bf16->fp32.
"""

import sys

sys.path.insert(0, "/opt/trn_rl_repo")

import numpy as np

# Problem constants
C = 64
H = 256
W = 256
FH = 3
FW = 3
RANK = 64
F = 128
HO = H - FH + 1  # 254
WO = W - FW + 1  # 254
NCORES = 8
ROWS = 32
IN_ROWS = ROWS + 2  # 34
NCOLS = ROWS * W  # 8192 output cols per core
XCOLS = IN_ROWS * W  # 8704 input cols (lower half)
XCOLS_UP = XCOLS - W  # 8448 input cols (upper half)
CHUNK = 512  # output cols per chunk (= 2 rows)
NCHUNK = NCOLS // CHUNK  # 16
NPAIR = NCHUNK // 2  # 8
SLOT = CHUNK + 8  # t2 slot width (pad for shifted reads)
W1COLS = 320  # [wa0;wa1](64) | [wb0;wb1](128) | [wa2;0](64) | unused(64)
XOFF = W1COLS + F  # 448: x data starts here
XALL = XOFF + XCOLS  # 9152

_PROGRAM_CACHE = {}


def _bf16():
    import ml_dtypes

    return np.dtype(ml_dtypes.bfloat16)


def build_program(num_devices=NCORES, reps=1, bench_internal=False, lag=4):
    """Per-core Bass program. lag = A->B software-pipeline distance in
    chunk-pairs."""
    from concourse import bacc, mybir, tile

    dt_c = mybir.dt.bfloat16
    dt_f32 = mybir.dt.float32

    nc = bacc.Bacc(
        "TRN2", target_bir_lowering=False, debug=False, num_devices=num_devices
    )
    if bench_internal:
        xall = nc.dram_tensor("xall_int", [2 * C, XALL], dt_c).ap()
        y = nc.dram_tensor("y_int", [F, NCOLS], dt_c).ap()
        tin = nc.dram_tensor("tin", [1, 16], dt_f32, kind="ExternalInput").ap()
        tout = nc.dram_tensor("tout", [1, 16], dt_f32, kind="ExternalOutput").ap()
    else:
        xall = nc.dram_tensor("xall", [2 * C, XALL], dt_c, kind="ExternalInput").ap()
        y = nc.dram_tensor("y", [F, NCOLS], dt_c, kind="ExternalOutput").ap()

    with tile.TileContext(nc) as tc:
        with (
            tc.tile_pool(name="xin", bufs=1) as xin_pool,
            tc.tile_pool(name="t2", bufs=lag + 1) as t2_pool,
            tc.tile_pool(name="ot", bufs=4) as ot_pool,
            tc.tile_pool(name="p1", bufs=3, space="PSUM") as p1_pool,
            tc.tile_pool(name="p2", bufs=3, space="PSUM") as p2_pool,
            tc.tile_pool(name="pw", bufs=1, space="PSUM") as pw_pool,
        ):

            def body():
                XA = xin_pool.tile([2 * C, XALL], dt_c)
                # one stream; small first piece (weights + 4 rows covers all
                # of chunk 0 + chunk 1's packed reads), then progressively
                # larger pieces that stay ahead of consumption.
                bnds = [
                    0,
                    XOFF + 4 * W,
                    XOFF + 7 * W,
                    XOFF + 14 * W,
                    XOFF + 24 * W,
                    XALL,
                ]
                for a, b in zip(bnds, bnds[1:]):
                    nc.sync.dma_start(out=XA[:, a:b], in_=xall[:, a:b])
                if bench_internal:
                    nc.sync.dma_start(out=tout[:], in_=tin[:])

                # PE p-state warm-up: dummy matmuls on a never-written scratch
                # tile fill the input-DMA wait so real matmuls start at full
                # clock (ramp needs ~3us of continuous PE busy). No deps.
                scr = xin_pool.tile([C, 256], dt_c, tag="warm")
                pw = pw_pool.tile([C, 256], dt_f32)
                nc.vector.memset(scr[:], 0.0)
                for _ in range(10):
                    nc.tensor.matmul(
                        out=pw[:],
                        lhsT=scr[:, 0:64],
                        rhs=scr[:],
                        start=True,
                        stop=True,
                    )

                WA01 = XA[:, 0:RANK]
                WB01 = XA[:, RANK : RANK + F]
                WA2 = XA[0:C, RANK + F : RANK + F + RANK]
                WB2 = XA[0:C, W1COLS : W1COLS + F]

                def stage_a(p):
                    # 2 chunks -> T2P pair tile; lower=t2, upper=t2 shifted +1
                    t2p = t2_pool.tile([2 * RANK, 2 * SLOT], dt_c, tag="t2")
                    t2v = t2p.rearrange("q (g s) -> q g s", s=SLOT)
                    # pad cols (read by shifted rhs APs, feed garbage outputs)
                    nc.vector.memset(t2v[0:RANK, :, CHUNK:SLOT], 0.0)
                    nc.vector.memset(t2v[RANK:, :, CHUNK - 1 : SLOT], 0.0)
                    for g in range(2):
                        b = XOFF + (2 * p + g) * CHUNK
                        p1 = p1_pool.tile([RANK, CHUNK], dt_f32)
                        nc.tensor.matmul(
                            out=p1[:],
                            lhsT=WA01,
                            rhs=XA[:, b : b + CHUNK],
                            start=True,
                            stop=False,
                        )
                        nc.tensor.matmul(
                            out=p1[:],
                            lhsT=WA2,
                            rhs=XA[0:C, b + 2 * W : b + 2 * W + CHUNK],
                            start=False,
                            stop=True,
                        )
                        s = g * SLOT
                        if g == 0:
                            nc.vector.tensor_copy(
                                out=t2p[0:RANK, s : s + CHUNK], in_=p1[:]
                            )
                        else:
                            nc.scalar.copy(out=t2p[0:RANK, s : s + CHUNK], in_=p1[:])
                    # dup: upper half = lower shifted +1 col (both slots)
                    nc.sync.dma_start(
                        out=t2v[RANK:, :, 0 : CHUNK - 1],
                        in_=t2v[0:RANK, :, 1:CHUNK],
                    )
                    return t2p

                def stage_b(p, t2p):
                    # final pair: store per chunk for a shorter tail
                    split = p == NPAIR - 1
                    last = split
                    ot = None if split else ot_pool.tile([F, 2 * CHUNK], dt_c)
                    for g in range(2):
                        p2 = p2_pool.tile([F, CHUNK], dt_f32)
                        nc.tensor.matmul(
                            out=p2[:],
                            lhsT=WB01,
                            rhs=t2p[:, g * SLOT : g * SLOT + CHUNK],
                            start=True,
                            stop=False,
                        )
                        nc.tensor.matmul(
                            out=p2[:],
                            lhsT=WB2,
                            rhs=t2p[0:RANK, g * SLOT + 2 : g * SLOT + 2 + CHUNK],
                            start=False,
                            stop=True,
                        )
                        b0 = (2 * p + g) * CHUNK
                        if split:
                            # per-chunk store on the (idle) sync queue; very
                            # last pair also splits the copy across engines
                            otg = ot_pool.tile([F, CHUNK], dt_c, tag="ot_s")
                            if last:
                                hc = CHUNK // 2
                                nc.scalar.copy(out=otg[:, 0:hc], in_=p2[:, 0:hc])
                                nc.vector.tensor_copy(
                                    out=otg[:, hc:CHUNK], in_=p2[:, hc:CHUNK]
                                )
                            elif g == 0:
                                nc.scalar.copy(out=otg[:], in_=p2[:])
                            else:
                                nc.vector.tensor_copy(out=otg[:], in_=p2[:])
                            nc.sync.dma_start(out=y[:, b0 : b0 + CHUNK], in_=otg[:])
                        elif g == 0:
                            nc.scalar.copy(out=ot[:, 0:CHUNK], in_=p2[:])
                        else:
                            nc.vector.tensor_copy(out=ot[:, CHUNK : 2 * CHUNK], in_=p2[:])
                    if not split:
                        b0 = p * 2 * CHUNK
                        # stores via SWDGE (gpsimd, idle) to keep HWDGE clear
                        # for dup DMAs; never on compute-engine queues (a
                        # dma_start holds that engine's SEQ ~630ns).
                        eng = nc.gpsimd if p < NPAIR - 2 else nc.sync
                        eng.dma_start(out=y[:, b0 : b0 + 2 * CHUNK], in_=ot[:])

                # software pipeline over pairs
                pend = []
                for p in range(NPAIR + lag):
                    if p < NPAIR:
                        pend.append((p, stage_a(p)))
                    if p >= lag:
                        bp, t2p = pend.pop(0)
                        stage_b(bp, t2p)

            if reps == 1:
                body()
            else:
                with tc.For_i(0, reps, 1):
                    body()

    nc.compile()
    return nc


def _get_program():
    if "v2" not in _PROGRAM_CACHE:
        _PROGRAM_CACHE["v2"] = build_program()
    return _PROGRAM_CACHE["v2"]


ROW_STARTS = [0, 32, 64, 96, 128, 160, 192, 222]


def make_in_maps(input, factor0, factor1, factor2, factor3):
    bf16 = _bf16()
    f0 = np.asarray(factor0, np.float32)
    f1 = np.asarray(factor1, np.float32)
    f2 = np.asarray(factor2, np.float32)
    f3 = np.asarray(factor3, np.float32)
    # wa[h][c,r] = f3[c,r]*f1[h,r];  wb[w][r,f] = f2[w,r]*f0[f,r]
    wa = f3[None, :, :] * f1[:, None, :]  # (3, C, R)
    wb = f2[:, :, None] * f0.T[None, :, :]  # (3, R, F)
    inp = np.asarray(input, np.float32)

    hdr = np.zeros((2 * C, XOFF), np.float32)
    hdr[0:C, 0:RANK] = wa[0]
    hdr[C:, 0:RANK] = wa[1]
    hdr[0:C, RANK : RANK + F] = wb[0]
    hdr[C:, RANK : RANK + F] = wb[1]
    hdr[0:C, RANK + F : RANK + F + RANK] = wa[2]
    hdr[0:C, W1COLS : W1COLS + F] = wb[2]
    hdr_bf = hdr.astype(bf16)

    maps = []
    for s in ROW_STARTS:
        xa = np.zeros((2 * C, XALL), bf16)
        xa[:, 0:XOFF] = hdr_bf
        xa[0:C, XOFF:] = inp[:, s : s + IN_ROWS, :].reshape(C, XCOLS).astype(bf16)
        xa[C:, XOFF : XOFF + XCOLS_UP] = (
            inp[:, s + 1 : s + IN_ROWS, :].reshape(C, XCOLS_UP).astype(bf16)
        )
        maps.append({"xall": np.ascontiguousarray(xa)})
    return maps


def kernel(input, factor0, factor1, factor2, factor3):
    from concourse.bass_utils import run_bass_kernel_spmd

    nc = _get_program()
    in_maps = make_in_maps(input, factor0, factor1, factor2, factor3)
    res = run_bass_kernel_spmd(nc, in_maps, list(range(NCORES))).results
    out = np.empty((F, HO, WO), np.float32)
    for i, s in enumerate(ROW_STARTS):
        ys = res[i]["y"].reshape(F, ROWS, W)[:, :, 0:WO].astype(np.float32)
        if i < NCORES - 1:
            out[:, s : s + ROWS, :] = ys
        else:
            out[:, 224:HO, :] = ys[:, 2:ROWS, :]
    return out
